# revision 17
# baseline (speedup 1.0000x reference)
"""Multi-head attention (B=2, S=2048, D=1024, H=16) on 8 NeuronCores.

Megatron tensor parallelism: core r owns heads 2r, 2r+1 (a 128-wide
slice of D). Wq/Wk/Wv column-parallel; output projection token-parallel
via one AllToAll per batch (128x256 fp16 blocks, normalization done
sender-side so the receive path feeds matmuls directly).

Schedule: the attention inner loop is ACT(exp)-bound. Each key tile
emits one [128,1024] two-bank score psum (two 512-col matmuls sharing
the stationary kT slice), ONE [128,1024] exp ACT, and two attnV
matmuls, software-pipelined with lookahead 1 so the PE always has a
score matmul in flight while ACT drains. Softmax normalization:
reciprocal of the ones-row sums (DVE) -> gpsimd partition_broadcast ->
fused psum*recip multiply (DVE), so the PE and ACT never touch it.
Batch-0 output-projection matmuls are drip-fed one per key tile into
batch-1's attention slack. A tiny warmup AllToAll absorbs launch skew.

DMA queues: inputs/weights/receive/stores on sync (HWDGE), staging on
vector, broadcasts + collectives on gpsimd.
"""

import sys

sys.path.insert(0, "/opt/trn_rl_repo")

import numpy as np

B, S, D, H, DK = 2, 2048, 1024, 16, 64
NCORES = 8
TOK = B * S            # 4096
DKC = D // NCORES      # 128 = 2 heads per core
TOKB = S // NCORES     # 256 tokens per core per batch
KT = D // 128          # 8 contraction tiles
SKT = S // 128         # 16 key tiles per batch

_cache = {}


def _build():
    from contextlib import ExitStack

    from concourse import bacc
    import concourse.mybir as mybir
    import concourse.tile as tile

    f32 = mybir.dt.float32
    f16 = mybir.dt.float16
    Act = mybir.ActivationFunctionType

    nc = bacc.Bacc(
        "TRN2", target_bir_lowering=False, debug=False,
        enable_asserts=False, num_devices=NCORES,
    )

    xqT = nc.dram_tensor("xqT", [D, TOK], f16, kind="ExternalInput").ap()
    xkT = nc.dram_tensor("xkT", [D, TOK], f16, kind="ExternalInput").ap()
    xvT = nc.dram_tensor("xvT", [D, TOK], f16, kind="ExternalInput").ap()
    wq = nc.dram_tensor("wq", [D, DKC], f16, kind="ExternalInput").ap()
    wk = nc.dram_tensor("wk", [D, DKC], f16, kind="ExternalInput").ap()
    wv = nc.dram_tensor("wv", [D, 130], f16, kind="ExternalInput").ap()
    wo = nc.dram_tensor("wo", [D, D], f16, kind="ExternalInput").ap()
    bq = nc.dram_tensor("bq", [DKC, 1], f32, kind="ExternalInput").ap()
    bk = nc.dram_tensor("bk", [DKC, 1], f32, kind="ExternalInput").ap()
    bv = nc.dram_tensor("bv", [1, 130], f16, kind="ExternalInput").ap()
    bo = nc.dram_tensor("bo", [1, D], f16, kind="ExternalInput").ap()
    out_ext = nc.dram_tensor("out", [2 * TOKB, D], f32, kind="ExternalOutput").ap()

    with tile.TileContext(nc) as tc, ExitStack() as ctx, \
            nc.allow_low_precision("fp16 matmul operands, fp32 psum accumulate"):
        wpool = ctx.enter_context(tc.tile_pool(name="w", bufs=1))
        xqpool = ctx.enter_context(tc.tile_pool(name="xq", bufs=8))
        xkpool = ctx.enter_context(tc.tile_pool(name="xk", bufs=8))
        xvpool = ctx.enter_context(tc.tile_pool(name="xv", bufs=8))
        qkpool = ctx.enter_context(tc.tile_pool(name="qk", bufs=1))
        vpool = ctx.enter_context(tc.tile_pool(name="v", bufs=1))
        ptpool = ctx.enter_context(tc.tile_pool(name="pt", bufs=5))
        lnpool = ctx.enter_context(tc.tile_pool(name="ln", bufs=1))
        spool = ctx.enter_context(tc.tile_pool(name="sum", bufs=2))
        rbpool = ctx.enter_context(tc.tile_pool(name="rb", bufs=2))
        rpool = ctx.enter_context(tc.tile_pool(name="recv", bufs=1))
        opool = ctx.enter_context(tc.tile_pool(name="o", bufs=2))
        ps_sc = ctx.enter_context(tc.tile_pool(name="pssc", bufs=2, space="PSUM"))
        ps_acc = ctx.enter_context(tc.tile_pool(name="psacc", bufs=2, space="PSUM"))
        dram = ctx.enter_context(tc.tile_pool(name="dram", bufs=1, space="DRAM"))

        # ---- warmup collective: absorb launch skew + link setup ----
        wusrc = dram.tile([8, 16], f32, tag="wusrc")
        wudst = dram.tile([8, 16], f32, tag="wudst")
        wuinit = wpool.tile([8, 16], f32, tag="wuinit")
        nc.gpsimd.memset(wuinit[:], 0.0)
        nc.sync.dma_start(wusrc[:], wuinit[:])
        nc.gpsimd.collective_compute(
            "AllToAll", mybir.AluOpType.bypass,
            replica_groups=[list(range(NCORES))],
            ins=[wusrc.opt()], outs=[wudst.opt()],
        )

        # ---- weights: consolidated DMAs on the scalar queue so input
        # loads own the sync queue from t=0 ----
        wq_t = wpool.tile([128, KT * DKC], f16, tag="wq")
        nc.scalar.dma_start(wq_t[:].rearrange("p (k m) -> p k m", k=KT),
                            wq.rearrange("(k p) m -> p k m", k=KT))
        wk_t = wpool.tile([128, KT * DKC], f16, tag="wk")
        nc.scalar.dma_start(wk_t[:].rearrange("p (k m) -> p k m", k=KT),
                            wk.rearrange("(k p) m -> p k m", k=KT))
        wv_t = wpool.tile([128, KT * 130], f16, tag="wv")
        nc.scalar.dma_start(wv_t[:].rearrange("p (k m) -> p k m", k=KT),
                            wv.rearrange("(k p) m -> p k m", k=KT))
        wo_t = wpool.tile([128, KT * D], f16, tag="wo")
        nc.scalar.dma_start(wo_t[:].rearrange("p (k m) -> p k m", k=KT),
                            wo.rearrange("(k p) m -> p k m", k=KT))
        bq_t = wpool.tile([DKC, 1], f32, tag="bq")
        nc.scalar.dma_start(bq_t[:], bq[:])
        bk_t = wpool.tile([DKC, 1], f32, tag="bk")
        nc.scalar.dma_start(bk_t[:], bk[:])
        bv_t = wpool.tile([1, 130], f16, tag="bv")
        nc.scalar.dma_start(bv_t[:], bv[:])
        bvb = wpool.tile([128, 130], f16, tag="bvb")
        nc.gpsimd.partition_broadcast(bvb[:], bv_t[:])
        bo_t = wpool.tile([1, D], f16, tag="bo")
        nc.scalar.dma_start(bo_t[:], bo[:])
        bob = wpool.tile([128, D], f16, tag="bob")
        nc.gpsimd.partition_broadcast(bob[:], bo_t[:])

        # ---- persistent per-batch tiles ----
        a2a_src = [dram.tile([NCORES * 128, TOKB], f16, tag=f"a2asrc{b}",
                             name=f"a2asrc{b}") for b in range(B)]
        a2a_dst = [dram.tile([NCORES * 128, TOKB], f16, tag=f"a2adst{b}",
                             name=f"a2adst{b}") for b in range(B)]
        ln_t = [lnpool.tile([128, S], f16, tag=f"ln{b}", name=f"ln{b}")
                for b in range(B)]
        rtile = [rpool.tile([128, KT * TOKB], f16, tag=f"r{b}",
                            name=f"r{b}") for b in range(B)]
        v_tiles = [[None] * SKT for _ in range(B)]
        qT = [None, None]
        kT = [None, None]

        GROUPS = [(sqg, h) for sqg in (0, 1) for h in (0, 1)]

        def emit_inputs(b):
            t0 = b * S
            xq_l, xk_l, xv_l = [], [], []
            for src, pool, lst in ((xqT, xqpool, xq_l), (xkT, xkpool, xk_l),
                                   (xvT, xvpool, xv_l)):
                for k in range(KT):
                    t = pool.tile([128, S], f16, tag="xt")
                    nc.sync.dma_start(t[:], src[k * 128:(k + 1) * 128,
                                                t0:t0 + S])
                    lst.append(t)
            return xq_l, xk_l, xv_l

        def emit_qk_proj(b, xq_l, xk_l):
            qT[b] = qkpool.tile([128, S], f16, tag=f"qT{b}", name=f"qT{b}")
            kT[b] = qkpool.tile([128, S], f16, tag=f"kT{b}", name=f"kT{b}")
            for x_l, w_t, bias_t, dst in ((xq_l, wq_t, bq_t, qT[b]),
                                          (xk_l, wk_t, bk_t, kT[b])):
                for blk in range(4):
                    if blk % 2 == 0:
                        ps = ps_sc.tile([128, 1024], f32, tag="sc")
                    half = ps[:, (blk % 2) * 512:(blk % 2) * 512 + 512]
                    for k in range(KT):
                        nc.tensor.matmul(
                            half, lhsT=w_t[:, k * DKC:(k + 1) * DKC],
                            rhs=x_l[k][:, blk * 512:(blk + 1) * 512],
                            start=(k == 0), stop=(k == KT - 1),
                        )
                    nc.vector.tensor_scalar_add(
                        dst[:, blk * 512:(blk + 1) * 512], half,
                        bias_t[:, 0:1])

        def emit_v_proj(b, xv_l):
            for mi in range(SKT):
                if mi % 2 == 0:
                    ps = ps_sc.tile([128, 1024], f32, tag="sc")
                sub = ps[:, (mi % 2) * 512:(mi % 2) * 512 + 130]
                for k in range(KT):
                    nc.tensor.matmul(
                        sub, lhsT=xv_l[k][:, mi * 128:(mi + 1) * 128],
                        rhs=wv_t[:, k * 130:(k + 1) * 130],
                        start=(k == 0), stop=(k == KT - 1),
                    )
                vt = vpool.tile([128, 130], f16, tag=f"v{b}_{mi}")
                nc.vector.tensor_add(vt[:], sub, bvb[:])
                v_tiles[b][mi] = vt

        def emit_score(b, g, sk):
            sqg, h = GROUPS[g]
            hp = h * 64
            sp = ps_sc.tile([128, 1024], f32, tag="sc")
            for j in (0, 1):
                nc.tensor.matmul(
                    sp[:, j * 512:(j + 1) * 512],
                    lhsT=kT[b][hp:hp + 64, sk * 128:(sk + 1) * 128],
                    rhs=qT[b][hp:hp + 64,
                              (sqg * 2 + j) * 512:(sqg * 2 + j + 1) * 512],
                    start=True, stop=True,
                )
            return sp

        def emit_group_tail(b, g, acc):
            """Sender-side softmax normalization + staging for group g."""
            sqg, h = GROUPS[g]
            hp = h * 64
            sf = spool.tile([1, 1024], f32, tag="sf")
            nc.vector.tensor_copy(sf[:], acc[64:65, :])
            # NOTE: reciprocal_approx_fast misbehaves on HW when its input
            # AP starts at a nonzero base partition, so feed it base-0 SBUF.
            rf = spool.tile([1, 1024], f32, tag="rf")
            nc.vector.reciprocal_approx_fast(rf[:], sf[:])
            rb = rbpool.tile([64, 1024], f32, tag="rb")
            nc.gpsimd.partition_broadcast(rb[:], rf[:])
            nc.vector.tensor_mul(
                ln_t[b][hp:hp + 64, sqg * 1024:(sqg + 1) * 1024],
                acc[0:64, :], rb[:])
            if h == 1:
                nc.gpsimd.dma_start(
                    a2a_src[b][sqg * 512:(sqg + 1) * 512, :]
                    .rearrange("(c p) t -> p c t", c=4),
                    ln_t[b][:, sqg * 1024:(sqg + 1) * 1024]
                    .rearrange("p (c t) -> p c t", c=4))

        def outproj_mms(b, m2):
            """Thunks for one out-proj block; psum allocated at first call
            so the pool rotation follows actual emission order."""
            cell = {}

            def mk(n2, k):
                def mm():
                    if "op" not in cell:
                        cell["op"] = ps_acc.tile([128, 1024], f32, tag="acc", name="op")
                    nc.tensor.matmul(
                        cell["op"][:, n2 * 512:(n2 + 1) * 512],
                        lhsT=rtile[b][:, k * TOKB + m2 * 128:
                                      k * TOKB + (m2 + 1) * 128],
                        rhs=wo_t[:, k * D + n2 * 512:k * D + (n2 + 1) * 512],
                        start=(k == 0), stop=(k == KT - 1),
                    )
                return mm

            thunks = [mk(n2, k) for n2 in (0, 1) for k in range(KT)]

            def fin():
                ot = opool.tile([128, 1024], f32, tag="ot")
                nc.vector.tensor_add(ot[:], cell["op"][:], bob[:])
                nc.sync.dma_start(
                    out_ext[b * TOKB + m2 * 128:b * TOKB + (m2 + 1) * 128, :],
                    ot[:])
            thunks.append(fin)
            return thunks

        def emit_attention(b, pe_filler):
            units = [(g, sk) for g in range(4) for sk in range(SKT)]
            sps = {0: emit_score(b, *units[0]), 1: emit_score(b, *units[1])}
            accs = {}
            for i, (g, sk) in enumerate(units):
                if g not in accs:
                    accs[g] = ps_acc.tile([128, 1024], f32, tag="acc", name="acc")
                sqg, h = GROUPS[g]
                pt = ptpool.tile([128, 1024], f16, tag="pt")
                nc.scalar.activation(pt[:], sps.pop(i)[:], Act.Exp,
                                     scale=0.125)
                for j in (0, 1):
                    nc.tensor.matmul(
                        accs[g][0:65, j * 512:(j + 1) * 512],
                        lhsT=v_tiles[b][sk][:, h * 65:h * 65 + 65],
                        rhs=pt[:, j * 512:(j + 1) * 512],
                        start=(sk == 0), stop=(sk == SKT - 1),
                    )
                if i + 2 < len(units):
                    sps[i + 2] = emit_score(b, *units[i + 2])
                # drip out-proj work into the ACT-bound slack; start after
                # the first group's accumulator has been fully read so the
                # borrowed ps_acc generations never stall the PE queue
                if pe_filler and i >= 18:
                    pe_filler.pop(0)()
                if sk == SKT - 1:
                    emit_group_tail(b, g, accs.pop(g))

        def emit_recv(b):
            nc.sync.dma_start(
                rtile[b][:].rearrange("p (k t) -> p k t", k=KT),
                a2a_dst[b].rearrange("(k p) t -> p k t", k=KT))

        # ================= emission =================
        for b in range(B):
            xq_l, xk_l, xv_l = emit_inputs(b)
            if b == 1:
                emit_recv(0)
            emit_qk_proj(b, xq_l, xk_l)
            emit_v_proj(b, xv_l)
            filler = []
            if b == 1:
                filler = outproj_mms(0, 0) + outproj_mms(0, 1)
            emit_attention(b, filler)
            while filler:
                filler.pop(0)()
            nc.gpsimd.collective_compute(
                "AllToAll", mybir.AluOpType.bypass,
                replica_groups=[list(range(NCORES))],
                ins=[a2a_src[b].opt()], outs=[a2a_dst[b].opt()],
            )

        # ---- tail: batch-1 receive + output projection ----
        emit_recv(1)
        for m2 in (0, 1):
            for f in outproj_mms(1, m2):
                f()

    nc.compile()
    return nc


def _get_nc():
    if "nc" not in _cache:
        _cache["nc"] = _build()
    return _cache["nc"]


def kernel(query, key, value, Wq, bq, Wk, bk, Wv, bv, Wo, bo, trace=False):
    from concourse.bass_utils import run_bass_kernel_spmd

    nc = _get_nc()

    q = np.ascontiguousarray(
        np.asarray(query, np.float32).reshape(TOK, D).T.astype(np.float16))
    k = np.ascontiguousarray(
        np.asarray(key, np.float32).reshape(TOK, D).T.astype(np.float16))
    v = np.ascontiguousarray(
        np.asarray(value, np.float32).reshape(TOK, D).T.astype(np.float16))
    Wq = np.asarray(Wq, np.float16)
    Wk = np.asarray(Wk, np.float16)
    Wv = np.asarray(Wv, np.float16)
    Wo = np.ascontiguousarray(np.asarray(Wo, np.float16))
    bo_h = np.ascontiguousarray(np.asarray(bo, np.float16)[None, :])

    in_maps = []
    for r in range(NCORES):
        sl = slice(r * DKC, (r + 1) * DKC)
        # wv/bv padded to 130 cols: col 64 and 129 carry the softmax-sum
        # ones column (weight 0, bias 1).
        wv_pad = np.zeros((D, 130), np.float16)
        wv_pad[:, 0:64] = Wv[:, r * DKC:r * DKC + 64]
        wv_pad[:, 65:129] = Wv[:, r * DKC + 64:(r + 1) * DKC]
        bv_pad = np.zeros((1, 130), np.float16)
        bv_pad[0, 0:64] = np.asarray(bv, np.float16)[r * DKC:r * DKC + 64]
        bv_pad[0, 65:129] = np.asarray(bv, np.float16)[r * DKC + 64:
                                                       (r + 1) * DKC]
        bv_pad[0, 64] = 1.0
        bv_pad[0, 129] = 1.0
        in_maps.append({
            "xqT": q, "xkT": k, "xvT": v,
            "wq": np.ascontiguousarray(Wq[:, sl]),
            "wk": np.ascontiguousarray(Wk[:, sl]),
            "wv": wv_pad,
            "wo": Wo,
            "bq": np.ascontiguousarray(np.asarray(bq, np.float32)[sl, None]),
            "bk": np.ascontiguousarray(np.asarray(bk, np.float32)[sl, None]),
            "bv": bv_pad,
            "bo": bo_h,
        })

    res = run_bass_kernel_spmd(nc, in_maps, list(range(NCORES)), trace=trace)
    _cache["last_results"] = res

    out = np.empty((B, S, D), np.float32)
    for c in range(NCORES):
        o = res.results[c]["out"]
        for b in range(B):
            out[b, c * TOKB:(c + 1) * TOKB] = o[b * TOKB:(b + 1) * TOKB]
    return out


# revision 18
# speedup vs baseline: 1.0089x; 1.0089x over previous
"""Multi-head attention (B=2, S=2048, D=1024, H=16) on 8 NeuronCores.

Megatron tensor parallelism: core r owns heads 2r, 2r+1 (a 128-wide
slice of D). Wq/Wk/Wv column-parallel; output projection token-parallel
via one AllToAll per batch (128x256 fp16 blocks, normalization done
sender-side so the receive path feeds matmuls directly).

Schedule: the attention inner loop is ACT(exp)-bound. Each key tile
emits one [128,1024] two-bank score psum (two 512-col matmuls sharing
the stationary kT slice), ONE [128,1024] exp ACT, and two attnV
matmuls, software-pipelined with lookahead 1 so the PE always has a
score matmul in flight while ACT drains. Softmax normalization:
reciprocal of the ones-row sums (DVE) -> gpsimd partition_broadcast ->
fused psum*recip multiply (DVE), so the PE and ACT never touch it.
Batch-0 output-projection matmuls are drip-fed one per key tile into
batch-1's attention slack. A tiny warmup AllToAll absorbs launch skew.

DMA queues: inputs/weights/receive/stores on sync (HWDGE), staging on
vector, broadcasts + collectives on gpsimd.
"""

import sys

sys.path.insert(0, "/opt/trn_rl_repo")

import numpy as np

B, S, D, H, DK = 2, 2048, 1024, 16, 64
NCORES = 8
TOK = B * S            # 4096
DKC = D // NCORES      # 128 = 2 heads per core
TOKB = S // NCORES     # 256 tokens per core per batch
KT = D // 128          # 8 contraction tiles
SKT = S // 128         # 16 key tiles per batch

_cache = {}


def _build():
    from contextlib import ExitStack

    from concourse import bacc
    import concourse.mybir as mybir
    import concourse.tile as tile

    f32 = mybir.dt.float32
    f16 = mybir.dt.float16
    Act = mybir.ActivationFunctionType

    nc = bacc.Bacc(
        "TRN2", target_bir_lowering=False, debug=False,
        enable_asserts=False, num_devices=NCORES,
    )

    xqT = nc.dram_tensor("xqT", [D, TOK], f16, kind="ExternalInput").ap()
    xkT = nc.dram_tensor("xkT", [D, TOK], f16, kind="ExternalInput").ap()
    xvT = nc.dram_tensor("xvT", [D, TOK], f16, kind="ExternalInput").ap()
    wq = nc.dram_tensor("wq", [D, DKC], f16, kind="ExternalInput").ap()
    wk = nc.dram_tensor("wk", [D, DKC], f16, kind="ExternalInput").ap()
    wv = nc.dram_tensor("wv", [D, 130], f16, kind="ExternalInput").ap()
    wo = nc.dram_tensor("wo", [D, D], f16, kind="ExternalInput").ap()
    bq = nc.dram_tensor("bq", [DKC, 1], f32, kind="ExternalInput").ap()
    bk = nc.dram_tensor("bk", [DKC, 1], f32, kind="ExternalInput").ap()
    bv = nc.dram_tensor("bv", [1, 130], f16, kind="ExternalInput").ap()
    bo = nc.dram_tensor("bo", [1, D], f16, kind="ExternalInput").ap()
    out_ext = nc.dram_tensor("out", [2 * TOKB, D], f32, kind="ExternalOutput").ap()

    with tile.TileContext(nc) as tc, ExitStack() as ctx, \
            nc.allow_low_precision("fp16 matmul operands, fp32 psum accumulate"):
        wpool = ctx.enter_context(tc.tile_pool(name="w", bufs=1))
        xqpool = ctx.enter_context(tc.tile_pool(name="xq", bufs=8))
        xkpool = ctx.enter_context(tc.tile_pool(name="xk", bufs=8))
        xvpool = ctx.enter_context(tc.tile_pool(name="xv", bufs=8))
        qkpool = ctx.enter_context(tc.tile_pool(name="qk", bufs=1))
        vpool = ctx.enter_context(tc.tile_pool(name="v", bufs=1))
        ptpool = ctx.enter_context(tc.tile_pool(name="pt", bufs=4))
        lnpool = ctx.enter_context(tc.tile_pool(name="ln", bufs=1))
        spool = ctx.enter_context(tc.tile_pool(name="sum", bufs=2))
        rbpool = ctx.enter_context(tc.tile_pool(name="rb", bufs=2))
        rpool = ctx.enter_context(tc.tile_pool(name="recv", bufs=1))
        opool = ctx.enter_context(tc.tile_pool(name="o", bufs=2))
        ps_sc = ctx.enter_context(tc.tile_pool(name="pssc", bufs=2, space="PSUM"))
        ps_acc = ctx.enter_context(tc.tile_pool(name="psacc", bufs=2, space="PSUM"))
        dram = ctx.enter_context(tc.tile_pool(name="dram", bufs=1, space="DRAM"))

        # ---- warmup collective: absorb launch skew + link setup ----
        wusrc = dram.tile([8, 16], f32, tag="wusrc")
        wudst = dram.tile([8, 16], f32, tag="wudst")
        wuinit = wpool.tile([8, 16], f32, tag="wuinit")
        nc.gpsimd.memset(wuinit[:], 0.0)
        nc.sync.dma_start(wusrc[:], wuinit[:])
        nc.gpsimd.collective_compute(
            "AllToAll", mybir.AluOpType.bypass,
            replica_groups=[list(range(NCORES))],
            ins=[wusrc.opt()], outs=[wudst.opt()],
        )

        # ---- weights: consolidated DMAs on the scalar queue so input
        # loads own the sync queue from t=0 ----
        wq_t = wpool.tile([128, KT * DKC], f16, tag="wq")
        nc.scalar.dma_start(wq_t[:].rearrange("p (k m) -> p k m", k=KT),
                            wq.rearrange("(k p) m -> p k m", k=KT))
        wk_t = wpool.tile([128, KT * DKC], f16, tag="wk")
        nc.scalar.dma_start(wk_t[:].rearrange("p (k m) -> p k m", k=KT),
                            wk.rearrange("(k p) m -> p k m", k=KT))
        wv_t = wpool.tile([128, KT * 130], f16, tag="wv")
        nc.scalar.dma_start(wv_t[:].rearrange("p (k m) -> p k m", k=KT),
                            wv.rearrange("(k p) m -> p k m", k=KT))
        wo_t = wpool.tile([128, KT * D], f16, tag="wo")
        nc.scalar.dma_start(wo_t[:].rearrange("p (k m) -> p k m", k=KT),
                            wo.rearrange("(k p) m -> p k m", k=KT))
        bq_t = wpool.tile([DKC, 1], f32, tag="bq")
        nc.scalar.dma_start(bq_t[:], bq[:])
        bk_t = wpool.tile([DKC, 1], f32, tag="bk")
        nc.scalar.dma_start(bk_t[:], bk[:])
        bv_t = wpool.tile([1, 130], f16, tag="bv")
        nc.scalar.dma_start(bv_t[:], bv[:])
        bvb = wpool.tile([128, 130], f16, tag="bvb")
        nc.gpsimd.partition_broadcast(bvb[:], bv_t[:])
        bo_t = wpool.tile([1, D], f16, tag="bo")
        nc.scalar.dma_start(bo_t[:], bo[:])
        bob = wpool.tile([128, D], f16, tag="bob")
        nc.gpsimd.partition_broadcast(bob[:], bo_t[:])

        # ---- persistent per-batch tiles ----
        a2a_src = [dram.tile([NCORES * 128, TOKB], f16, tag=f"a2asrc{b}",
                             name=f"a2asrc{b}") for b in range(B)]
        a2a_dst = [dram.tile([NCORES * 128, TOKB], f16, tag=f"a2adst{b}",
                             name=f"a2adst{b}") for b in range(B)]
        ln_t = [lnpool.tile([128, S], f16, tag=f"ln{b}", name=f"ln{b}")
                for b in range(B)]
        rtile = [rpool.tile([128, KT * TOKB], f16, tag=f"r{b}",
                            name=f"r{b}") for b in range(B)]
        v_tiles = [[None] * SKT for _ in range(B)]
        qT = [None, None]
        kT = [None, None]

        GROUPS = [(sqg, h) for sqg in (0, 1) for h in (0, 1)]

        def emit_inputs(b):
            t0 = b * S
            xq_l, xk_l, xv_l = [], [], []
            for src, pool, lst in ((xqT, xqpool, xq_l), (xkT, xkpool, xk_l),
                                   (xvT, xvpool, xv_l)):
                for k in range(KT):
                    t = pool.tile([128, S], f16, tag="xt")
                    nc.sync.dma_start(t[:], src[k * 128:(k + 1) * 128,
                                                t0:t0 + S])
                    lst.append(t)
            return xq_l, xk_l, xv_l

        def emit_qk_proj(b, xq_l, xk_l):
            qT[b] = qkpool.tile([128, S], f16, tag=f"qT{b}", name=f"qT{b}")
            kT[b] = qkpool.tile([128, S], f16, tag=f"kT{b}", name=f"kT{b}")
            for x_l, w_t, bias_t, dst in ((xq_l, wq_t, bq_t, qT[b]),
                                          (xk_l, wk_t, bk_t, kT[b])):
                for blk in range(4):
                    if blk % 2 == 0:
                        ps = ps_sc.tile([128, 1024], f32, tag="sc")
                    half = ps[:, (blk % 2) * 512:(blk % 2) * 512 + 512]
                    for k in range(KT):
                        nc.tensor.matmul(
                            half, lhsT=w_t[:, k * DKC:(k + 1) * DKC],
                            rhs=x_l[k][:, blk * 512:(blk + 1) * 512],
                            start=(k == 0), stop=(k == KT - 1),
                        )
                    nc.vector.tensor_scalar_add(
                        dst[:, blk * 512:(blk + 1) * 512], half,
                        bias_t[:, 0:1])

        def emit_v_proj(b, xv_l):
            for mi in range(SKT):
                if mi % 2 == 0:
                    ps = ps_sc.tile([128, 1024], f32, tag="sc")
                sub = ps[:, (mi % 2) * 512:(mi % 2) * 512 + 130]
                for k in range(KT):
                    nc.tensor.matmul(
                        sub, lhsT=xv_l[k][:, mi * 128:(mi + 1) * 128],
                        rhs=wv_t[:, k * 130:(k + 1) * 130],
                        start=(k == 0), stop=(k == KT - 1),
                    )
                vt = vpool.tile([128, 130], f16, tag=f"v{b}_{mi}")
                nc.vector.tensor_add(vt[:], sub, bvb[:])
                v_tiles[b][mi] = vt

        def emit_score(b, g, sk):
            sqg, h = GROUPS[g]
            hp = h * 64
            sp = ps_sc.tile([128, 1024], f32, tag="sc")
            for j in (0, 1):
                nc.tensor.matmul(
                    sp[:, j * 512:(j + 1) * 512],
                    lhsT=kT[b][hp:hp + 64, sk * 128:(sk + 1) * 128],
                    rhs=qT[b][hp:hp + 64,
                              (sqg * 2 + j) * 512:(sqg * 2 + j + 1) * 512],
                    start=True, stop=True,
                )
            return sp

        def emit_group_tail(b, g, acc):
            """Sender-side softmax normalization + staging for group g."""
            sqg, h = GROUPS[g]
            hp = h * 64
            sf = spool.tile([1, 1024], f32, tag="sf")
            nc.vector.tensor_copy(sf[:], acc[64:65, :])
            # NOTE: reciprocal_approx_fast misbehaves on HW when its input
            # AP starts at a nonzero base partition, so feed it base-0 SBUF.
            rf = spool.tile([1, 1024], f32, tag="rf")
            nc.vector.reciprocal_approx_fast(rf[:], sf[:])
            rb = rbpool.tile([64, 1024], f32, tag="rb")
            nc.gpsimd.partition_broadcast(rb[:], rf[:])
            nc.vector.tensor_mul(
                ln_t[b][hp:hp + 64, sqg * 1024:(sqg + 1) * 1024],
                acc[0:64, :], rb[:])
            if h == 1:
                nc.gpsimd.dma_start(
                    a2a_src[b][sqg * 512:(sqg + 1) * 512, :]
                    .rearrange("(c p) t -> p c t", c=4),
                    ln_t[b][:, sqg * 1024:(sqg + 1) * 1024]
                    .rearrange("p (c t) -> p c t", c=4))

        def outproj_mms(b, m2):
            """Thunks for one out-proj block; psum allocated at first call
            so the pool rotation follows actual emission order."""
            cell = {}

            def mk(n2, k):
                def mm():
                    if "op" not in cell:
                        cell["op"] = ps_acc.tile([128, 1024], f32, tag="acc", name="op")
                    nc.tensor.matmul(
                        cell["op"][:, n2 * 512:(n2 + 1) * 512],
                        lhsT=rtile[b][:, k * TOKB + m2 * 128:
                                      k * TOKB + (m2 + 1) * 128],
                        rhs=wo_t[:, k * D + n2 * 512:k * D + (n2 + 1) * 512],
                        start=(k == 0), stop=(k == KT - 1),
                    )
                return mm

            thunks = [mk(n2, k) for n2 in (0, 1) for k in range(KT)]

            def fin():
                ot = opool.tile([128, 1024], f32, tag="ot")
                nc.vector.tensor_add(ot[:], cell["op"][:], bob[:])
                nc.sync.dma_start(
                    out_ext[b * TOKB + m2 * 128:b * TOKB + (m2 + 1) * 128, :],
                    ot[:])
            thunks.append(fin)
            return thunks

        def emit_attention(b, pe_filler):
            units = [(g, sk) for g in range(4) for sk in range(SKT)]
            sps = {0: emit_score(b, *units[0]), 1: emit_score(b, *units[1])}
            accs = {}
            for i, (g, sk) in enumerate(units):
                if g not in accs:
                    accs[g] = ps_acc.tile([128, 1024], f32, tag="acc", name="acc")
                sqg, h = GROUPS[g]
                pt = ptpool.tile([128, 1024], f16, tag="pt")
                nc.scalar.activation(pt[:], sps.pop(i)[:], Act.Exp,
                                     scale=0.125)
                for j in (0, 1):
                    nc.tensor.matmul(
                        accs[g][0:65, j * 512:(j + 1) * 512],
                        lhsT=v_tiles[b][sk][:, h * 65:h * 65 + 65],
                        rhs=pt[:, j * 512:(j + 1) * 512],
                        start=(sk == 0), stop=(sk == SKT - 1),
                    )
                if i + 2 < len(units):
                    sps[i + 2] = emit_score(b, *units[i + 2])
                # drip out-proj work into the ACT-bound slack; start after
                # the first group's accumulator has been fully read so the
                # borrowed ps_acc generations never stall the PE queue
                if pe_filler and i >= 18:
                    pe_filler.pop(0)()
                if sk == SKT - 1:
                    emit_group_tail(b, g, accs.pop(g))

        def emit_recv(b):
            nc.sync.dma_start(
                rtile[b][:].rearrange("p (k t) -> p k t", k=KT),
                a2a_dst[b].rearrange("(k p) t -> p k t", k=KT))

        # ================= emission =================
        for b in range(B):
            xq_l, xk_l, xv_l = emit_inputs(b)
            if b == 1:
                emit_recv(0)
            emit_qk_proj(b, xq_l, xk_l)
            emit_v_proj(b, xv_l)
            filler = []
            if b == 1:
                filler = outproj_mms(0, 0) + outproj_mms(0, 1)
            emit_attention(b, filler)
            while filler:
                filler.pop(0)()
            nc.gpsimd.collective_compute(
                "AllToAll", mybir.AluOpType.bypass,
                replica_groups=[list(range(NCORES))],
                ins=[a2a_src[b].opt()], outs=[a2a_dst[b].opt()],
            )

        # ---- tail: batch-1 receive + output projection ----
        emit_recv(1)
        for m2 in (0, 1):
            for f in outproj_mms(1, m2):
                f()

    nc.compile()
    return nc


def _get_nc():
    if "nc" not in _cache:
        _cache["nc"] = _build()
    return _cache["nc"]


def kernel(query, key, value, Wq, bq, Wk, bk, Wv, bv, Wo, bo, trace=False):
    from concourse.bass_utils import run_bass_kernel_spmd

    nc = _get_nc()

    q = np.ascontiguousarray(
        np.asarray(query, np.float32).reshape(TOK, D).T.astype(np.float16))
    k = np.ascontiguousarray(
        np.asarray(key, np.float32).reshape(TOK, D).T.astype(np.float16))
    v = np.ascontiguousarray(
        np.asarray(value, np.float32).reshape(TOK, D).T.astype(np.float16))
    Wq = np.asarray(Wq, np.float16)
    Wk = np.asarray(Wk, np.float16)
    Wv = np.asarray(Wv, np.float16)
    Wo = np.ascontiguousarray(np.asarray(Wo, np.float16))
    bo_h = np.ascontiguousarray(np.asarray(bo, np.float16)[None, :])

    in_maps = []
    for r in range(NCORES):
        sl = slice(r * DKC, (r + 1) * DKC)
        # wv/bv padded to 130 cols: col 64 and 129 carry the softmax-sum
        # ones column (weight 0, bias 1).
        wv_pad = np.zeros((D, 130), np.float16)
        wv_pad[:, 0:64] = Wv[:, r * DKC:r * DKC + 64]
        wv_pad[:, 65:129] = Wv[:, r * DKC + 64:(r + 1) * DKC]
        bv_pad = np.zeros((1, 130), np.float16)
        bv_pad[0, 0:64] = np.asarray(bv, np.float16)[r * DKC:r * DKC + 64]
        bv_pad[0, 65:129] = np.asarray(bv, np.float16)[r * DKC + 64:
                                                       (r + 1) * DKC]
        bv_pad[0, 64] = 1.0
        bv_pad[0, 129] = 1.0
        in_maps.append({
            "xqT": q, "xkT": k, "xvT": v,
            "wq": np.ascontiguousarray(Wq[:, sl]),
            "wk": np.ascontiguousarray(Wk[:, sl]),
            "wv": wv_pad,
            "wo": Wo,
            "bq": np.ascontiguousarray(np.asarray(bq, np.float32)[sl, None]),
            "bk": np.ascontiguousarray(np.asarray(bk, np.float32)[sl, None]),
            "bv": bv_pad,
            "bo": bo_h,
        })

    res = run_bass_kernel_spmd(nc, in_maps, list(range(NCORES)), trace=trace)
    _cache["last_results"] = res

    out = np.empty((B, S, D), np.float32)
    for c in range(NCORES):
        o = res.results[c]["out"]
        for b in range(B):
            out[b, c * TOKB:(c + 1) * TOKB] = o[b * TOKB:(b + 1) * TOKB]
    return out


# revision 19
# speedup vs baseline: 1.0133x; 1.0044x over previous
"""Multi-head attention (B=2, S=2048, D=1024, H=16) on 8 NeuronCores.

Megatron tensor parallelism: core r owns heads 2r, 2r+1 (a 128-wide
slice of D). Wq/Wk/Wv column-parallel; output projection token-parallel
via one AllToAll per batch (128x256 fp16 blocks, normalization done
sender-side so the receive path feeds matmuls directly).

Schedule: the attention inner loop is ACT(exp)-bound. Each key tile
emits one [128,1024] two-bank score psum (two 512-col matmuls sharing
the stationary kT slice), ONE [128,1024] exp ACT, and two attnV
matmuls, software-pipelined with lookahead 1 so the PE always has a
score matmul in flight while ACT drains. Softmax normalization:
reciprocal of the ones-row sums (DVE) -> gpsimd partition_broadcast ->
fused psum*recip multiply (DVE), so the PE and ACT never touch it.
Batch-0 output-projection matmuls are drip-fed one per key tile into
batch-1's attention slack. A tiny warmup AllToAll absorbs launch skew.

DMA queues: inputs/weights/receive/stores on sync (HWDGE), staging on
vector, broadcasts + collectives on gpsimd.
"""

import sys

sys.path.insert(0, "/opt/trn_rl_repo")

import numpy as np

B, S, D, H, DK = 2, 2048, 1024, 16, 64
NCORES = 8
TOK = B * S            # 4096
DKC = D // NCORES      # 128 = 2 heads per core
TOKB = S // NCORES     # 256 tokens per core per batch
KT = D // 128          # 8 contraction tiles
SKT = S // 128         # 16 key tiles per batch

_cache = {}


def _build():
    from contextlib import ExitStack

    from concourse import bacc
    import concourse.mybir as mybir
    import concourse.tile as tile

    f32 = mybir.dt.float32
    f16 = mybir.dt.float16
    Act = mybir.ActivationFunctionType

    nc = bacc.Bacc(
        "TRN2", target_bir_lowering=False, debug=False,
        enable_asserts=False, num_devices=NCORES,
    )

    xqT = nc.dram_tensor("xqT", [D, TOK], f16, kind="ExternalInput").ap()
    xkT = nc.dram_tensor("xkT", [D, TOK], f16, kind="ExternalInput").ap()
    xvT = nc.dram_tensor("xvT", [D, TOK], f16, kind="ExternalInput").ap()
    wq = nc.dram_tensor("wq", [D, DKC], f16, kind="ExternalInput").ap()
    wk = nc.dram_tensor("wk", [D, DKC], f16, kind="ExternalInput").ap()
    wv = nc.dram_tensor("wv", [D, 130], f16, kind="ExternalInput").ap()
    wo = nc.dram_tensor("wo", [D, D], f16, kind="ExternalInput").ap()
    bq = nc.dram_tensor("bq", [DKC, 1], f32, kind="ExternalInput").ap()
    bk = nc.dram_tensor("bk", [DKC, 1], f32, kind="ExternalInput").ap()
    bv = nc.dram_tensor("bv", [1, 130], f16, kind="ExternalInput").ap()
    bo = nc.dram_tensor("bo", [1, D], f16, kind="ExternalInput").ap()
    out_ext = nc.dram_tensor("out", [2 * TOKB, D], f32, kind="ExternalOutput").ap()

    with tile.TileContext(nc) as tc, ExitStack() as ctx, \
            nc.allow_low_precision("fp16 matmul operands, fp32 psum accumulate"):
        wpool = ctx.enter_context(tc.tile_pool(name="w", bufs=1))
        xqpool = ctx.enter_context(tc.tile_pool(name="xq", bufs=8))
        xkpool = ctx.enter_context(tc.tile_pool(name="xk", bufs=8))
        xvpool = ctx.enter_context(tc.tile_pool(name="xv", bufs=8))
        qkpool = ctx.enter_context(tc.tile_pool(name="qk", bufs=1))
        vpool = ctx.enter_context(tc.tile_pool(name="v", bufs=1))
        ptpool = ctx.enter_context(tc.tile_pool(name="pt", bufs=4))
        lnpool = ctx.enter_context(tc.tile_pool(name="ln", bufs=1))
        spool = ctx.enter_context(tc.tile_pool(name="sum", bufs=2))
        rbpool = ctx.enter_context(tc.tile_pool(name="rb", bufs=2))
        rpool = ctx.enter_context(tc.tile_pool(name="recv", bufs=1))
        opool = ctx.enter_context(tc.tile_pool(name="o", bufs=2))
        ps_sc = ctx.enter_context(tc.tile_pool(name="pssc", bufs=2, space="PSUM"))
        ps_acc = ctx.enter_context(tc.tile_pool(name="psacc", bufs=2, space="PSUM"))
        dram = ctx.enter_context(tc.tile_pool(name="dram", bufs=1, space="DRAM"))

        # ---- warmup collective: absorb launch skew + link setup ----
        wusrc = dram.tile([8, 16], f32, tag="wusrc")
        wudst = dram.tile([8, 16], f32, tag="wudst")
        wuinit = wpool.tile([8, 16], f32, tag="wuinit")
        nc.gpsimd.memset(wuinit[:], 0.0)
        nc.sync.dma_start(wusrc[:], wuinit[:])
        nc.gpsimd.collective_compute(
            "AllToAll", mybir.AluOpType.bypass,
            replica_groups=[list(range(NCORES))],
            ins=[wusrc.opt()], outs=[wudst.opt()],
        )

        # ---- weights: consolidated DMAs on the scalar queue so input
        # loads own the sync queue from t=0 ----
        wq_t = wpool.tile([128, KT * DKC], f16, tag="wq")
        nc.scalar.dma_start(wq_t[:].rearrange("p (k m) -> p k m", k=KT),
                            wq.rearrange("(k p) m -> p k m", k=KT))
        wk_t = wpool.tile([128, KT * DKC], f16, tag="wk")
        nc.scalar.dma_start(wk_t[:].rearrange("p (k m) -> p k m", k=KT),
                            wk.rearrange("(k p) m -> p k m", k=KT))
        wv_t = wpool.tile([128, KT * 130], f16, tag="wv")
        nc.scalar.dma_start(wv_t[:].rearrange("p (k m) -> p k m", k=KT),
                            wv.rearrange("(k p) m -> p k m", k=KT))
        wo_t = wpool.tile([128, KT * D], f16, tag="wo")
        nc.scalar.dma_start(wo_t[:].rearrange("p (k m) -> p k m", k=KT),
                            wo.rearrange("(k p) m -> p k m", k=KT))
        bq_t = wpool.tile([DKC, 1], f32, tag="bq")
        nc.scalar.dma_start(bq_t[:], bq[:])
        bk_t = wpool.tile([DKC, 1], f32, tag="bk")
        nc.scalar.dma_start(bk_t[:], bk[:])
        bv_t = wpool.tile([1, 130], f16, tag="bv")
        nc.scalar.dma_start(bv_t[:], bv[:])
        bvb = wpool.tile([128, 130], f16, tag="bvb")
        nc.gpsimd.partition_broadcast(bvb[:], bv_t[:])
        bo_t = wpool.tile([1, D], f16, tag="bo")
        nc.scalar.dma_start(bo_t[:], bo[:])
        bob = wpool.tile([128, D], f16, tag="bob")
        nc.gpsimd.partition_broadcast(bob[:], bo_t[:])

        # ---- persistent per-batch tiles ----
        a2a_src = [dram.tile([NCORES * 128, TOKB], f16, tag=f"a2asrc{b}",
                             name=f"a2asrc{b}") for b in range(B)]
        a2a_dst = [dram.tile([NCORES * 128, TOKB], f16, tag=f"a2adst{b}",
                             name=f"a2adst{b}") for b in range(B)]
        ln_t = [lnpool.tile([128, S], f16, tag=f"ln{b}", name=f"ln{b}")
                for b in range(B)]
        rtile = [rpool.tile([128, KT * TOKB], f16, tag=f"r{b}",
                            name=f"r{b}") for b in range(B)]
        v_tiles = [[None] * SKT for _ in range(B)]
        qT = [None, None]
        kT = [None, None]

        GROUPS = [(sqg, h) for sqg in (0, 1) for h in (0, 1)]

        def emit_inputs(b):
            t0 = b * S
            xq_l, xk_l, xv_l = [], [], []
            for src, pool, lst in ((xqT, xqpool, xq_l), (xkT, xkpool, xk_l),
                                   (xvT, xvpool, xv_l)):
                for k in range(KT):
                    t = pool.tile([128, S], f16, tag="xt")
                    nc.sync.dma_start(t[:], src[k * 128:(k + 1) * 128,
                                                t0:t0 + S])
                    lst.append(t)
            return xq_l, xk_l, xv_l

        def emit_qk_proj(b, xq_l, xk_l):
            qT[b] = qkpool.tile([128, S], f16, tag=f"qT{b}", name=f"qT{b}")
            kT[b] = qkpool.tile([128, S], f16, tag=f"kT{b}", name=f"kT{b}")
            for x_l, w_t, bias_t, dst in ((xq_l, wq_t, bq_t, qT[b]),
                                          (xk_l, wk_t, bk_t, kT[b])):
                for blk in range(4):
                    if blk % 2 == 0:
                        ps = ps_sc.tile([128, 1024], f32, tag="sc")
                    half = ps[:, (blk % 2) * 512:(blk % 2) * 512 + 512]
                    for k in range(KT):
                        nc.tensor.matmul(
                            half, lhsT=w_t[:, k * DKC:(k + 1) * DKC],
                            rhs=x_l[k][:, blk * 512:(blk + 1) * 512],
                            start=(k == 0), stop=(k == KT - 1),
                        )
                    nc.vector.tensor_scalar_add(
                        dst[:, blk * 512:(blk + 1) * 512], half,
                        bias_t[:, 0:1])

        def emit_v_proj(b, xv_l):
            for mi in range(SKT):
                if mi % 2 == 0:
                    ps = ps_sc.tile([128, 1024], f32, tag="sc")
                sub = ps[:, (mi % 2) * 512:(mi % 2) * 512 + 130]
                for k in range(KT):
                    nc.tensor.matmul(
                        sub, lhsT=xv_l[k][:, mi * 128:(mi + 1) * 128],
                        rhs=wv_t[:, k * 130:(k + 1) * 130],
                        start=(k == 0), stop=(k == KT - 1),
                    )
                vt = vpool.tile([128, 130], f16, tag=f"v{b}_{mi}")
                nc.vector.tensor_add(vt[:], sub, bvb[:])
                v_tiles[b][mi] = vt

        def emit_score(b, g, sk):
            sqg, h = GROUPS[g]
            hp = h * 64
            sp = ps_sc.tile([128, 1024], f32, tag="sc")
            for j in (0, 1):
                nc.tensor.matmul(
                    sp[:, j * 512:(j + 1) * 512],
                    lhsT=kT[b][hp:hp + 64, sk * 128:(sk + 1) * 128],
                    rhs=qT[b][hp:hp + 64,
                              (sqg * 2 + j) * 512:(sqg * 2 + j + 1) * 512],
                    start=True, stop=True,
                )
            return sp

        def emit_group_tail(b, g, acc):
            """Sender-side softmax normalization + staging for group g."""
            sqg, h = GROUPS[g]
            hp = h * 64
            sf = spool.tile([1, 1024], f32, tag="sf")
            nc.vector.tensor_copy(sf[:], acc[64:65, :])
            # NOTE: reciprocal_approx_fast misbehaves on HW when its input
            # AP starts at a nonzero base partition, so feed it base-0 SBUF.
            rf = spool.tile([1, 1024], f32, tag="rf")
            nc.vector.reciprocal_approx_fast(rf[:], sf[:])
            rb = rbpool.tile([64, 1024], f32, tag="rb")
            nc.gpsimd.partition_broadcast(rb[:], rf[:])
            nc.vector.tensor_mul(
                ln_t[b][hp:hp + 64, sqg * 1024:(sqg + 1) * 1024],
                acc[0:64, :], rb[:])
            if h == 1:
                nc.gpsimd.dma_start(
                    a2a_src[b][sqg * 512:(sqg + 1) * 512, :]
                    .rearrange("(c p) t -> p c t", c=4),
                    ln_t[b][:, sqg * 1024:(sqg + 1) * 1024]
                    .rearrange("p (c t) -> p c t", c=4))

        def qkproj_thunks(b, xq_l, xk_l):
            """q/k projection as filler thunks; psum gens allocated
            lazily from the acc tag so rotation follows emission order."""
            qT[b] = qkpool.tile([128, S], f16, tag=f"qT{b}", name=f"qT{b}")
            kT[b] = qkpool.tile([128, S], f16, tag=f"kT{b}", name=f"kT{b}")
            thunks = []
            for x_l, w_t, bias_t, dst in ((xq_l, wq_t, bq_t, qT[b]),
                                          (xk_l, wk_t, bk_t, kT[b])):
                for pair in range(2):
                    cell = {}
                    for blk in (pair * 2, pair * 2 + 1):
                        for k in range(KT):
                            def mm(blk=blk, k=k, cell=cell, x_l=x_l,
                                   w_t=w_t):
                                if "ps" not in cell:
                                    cell["ps"] = ps_acc.tile(
                                        [128, 1024], f32, tag="acc",
                                        name="pjps")
                                nc.tensor.matmul(
                                    cell["ps"][:, (blk % 2) * 512:
                                               (blk % 2) * 512 + 512],
                                    lhsT=w_t[:, k * DKC:(k + 1) * DKC],
                                    rhs=x_l[k][:, blk * 512:(blk + 1) * 512],
                                    start=(k == 0), stop=(k == KT - 1),
                                )
                            thunks.append(mm)

                        def bias(blk=blk, cell=cell, bias_t=bias_t, dst=dst):
                            nc.vector.tensor_scalar_add(
                                dst[:, blk * 512:(blk + 1) * 512],
                                cell["ps"][:, (blk % 2) * 512:
                                           (blk % 2) * 512 + 512],
                                bias_t[:, 0:1])
                        thunks.append(bias)
            return thunks

        def outproj_mms(b, m2):
            """Thunks for one out-proj block; psum allocated at first call
            so the pool rotation follows actual emission order."""
            cell = {}

            def mk(n2, k):
                def mm():
                    if "op" not in cell:
                        cell["op"] = ps_acc.tile([128, 1024], f32, tag="acc", name="op")
                    nc.tensor.matmul(
                        cell["op"][:, n2 * 512:(n2 + 1) * 512],
                        lhsT=rtile[b][:, k * TOKB + m2 * 128:
                                      k * TOKB + (m2 + 1) * 128],
                        rhs=wo_t[:, k * D + n2 * 512:k * D + (n2 + 1) * 512],
                        start=(k == 0), stop=(k == KT - 1),
                    )
                return mm

            thunks = [mk(n2, k) for n2 in (0, 1) for k in range(KT)]

            def fin():
                ot = opool.tile([128, 1024], f32, tag="ot")
                nc.vector.tensor_add(ot[:], cell["op"][:], bob[:])
                nc.sync.dma_start(
                    out_ext[b * TOKB + m2 * 128:b * TOKB + (m2 + 1) * 128, :],
                    ot[:])
            thunks.append(fin)
            return thunks

        def emit_attention(b, pe_filler):
            units = [(g, sk) for g in range(4) for sk in range(SKT)]
            sps = {0: emit_score(b, *units[0]), 1: emit_score(b, *units[1])}
            accs = {}
            for i, (g, sk) in enumerate(units):
                if g not in accs:
                    accs[g] = ps_acc.tile([128, 1024], f32, tag="acc", name="acc")
                sqg, h = GROUPS[g]
                pt = ptpool.tile([128, 1024], f16, tag="pt")
                nc.scalar.activation(pt[:], sps.pop(i)[:], Act.Exp,
                                     scale=0.125)
                for j in (0, 1):
                    nc.tensor.matmul(
                        accs[g][0:65, j * 512:(j + 1) * 512],
                        lhsT=v_tiles[b][sk][:, h * 65:h * 65 + 65],
                        rhs=pt[:, j * 512:(j + 1) * 512],
                        start=(sk == 0), stop=(sk == SKT - 1),
                    )
                if i + 2 < len(units):
                    sps[i + 2] = emit_score(b, *units[i + 2])
                # drip out-proj work into the ACT-bound slack; start after
                # the first group's accumulator has been fully read so the
                # borrowed ps_acc generations never stall the PE queue
                if pe_filler and i >= 17:
                    pe_filler.pop(0)()
                if sk == SKT - 1:
                    emit_group_tail(b, g, accs.pop(g))

        def emit_recv(b):
            nc.sync.dma_start(
                rtile[b][:].rearrange("p (k t) -> p k t", k=KT),
                a2a_dst[b].rearrange("(k p) t -> p k t", k=KT))

        # ================= emission =================
        def emit_a2a(b):
            nc.gpsimd.collective_compute(
                "AllToAll", mybir.AluOpType.bypass,
                replica_groups=[list(range(NCORES))],
                ins=[a2a_src[b].opt()], outs=[a2a_dst[b].opt()],
            )

        xq0, xk0, xv0 = emit_inputs(0)
        xq1, xk1, xv1 = emit_inputs(1)
        emit_qk_proj(0, xq0, xk0)
        emit_v_proj(0, xv0)
        f0 = qkproj_thunks(1, xq1, xk1)
        emit_attention(0, f0)
        while f0:
            f0.pop(0)()
        emit_a2a(0)
        emit_recv(0)
        emit_v_proj(1, xv1)
        f1 = outproj_mms(0, 0) + outproj_mms(0, 1)
        emit_attention(1, f1)
        while f1:
            f1.pop(0)()
        emit_a2a(1)

        # ---- tail: batch-1 receive + output projection ----
        emit_recv(1)
        for m2 in (0, 1):
            for f in outproj_mms(1, m2):
                f()

    nc.compile()
    return nc


def _get_nc():
    if "nc" not in _cache:
        _cache["nc"] = _build()
    return _cache["nc"]


def kernel(query, key, value, Wq, bq, Wk, bk, Wv, bv, Wo, bo, trace=False):
    from concourse.bass_utils import run_bass_kernel_spmd

    nc = _get_nc()

    q = np.ascontiguousarray(
        np.asarray(query, np.float32).reshape(TOK, D).T.astype(np.float16))
    k = np.ascontiguousarray(
        np.asarray(key, np.float32).reshape(TOK, D).T.astype(np.float16))
    v = np.ascontiguousarray(
        np.asarray(value, np.float32).reshape(TOK, D).T.astype(np.float16))
    Wq = np.asarray(Wq, np.float16)
    Wk = np.asarray(Wk, np.float16)
    Wv = np.asarray(Wv, np.float16)
    Wo = np.ascontiguousarray(np.asarray(Wo, np.float16))
    bo_h = np.ascontiguousarray(np.asarray(bo, np.float16)[None, :])

    in_maps = []
    for r in range(NCORES):
        sl = slice(r * DKC, (r + 1) * DKC)
        # wv/bv padded to 130 cols: col 64 and 129 carry the softmax-sum
        # ones column (weight 0, bias 1).
        wv_pad = np.zeros((D, 130), np.float16)
        wv_pad[:, 0:64] = Wv[:, r * DKC:r * DKC + 64]
        wv_pad[:, 65:129] = Wv[:, r * DKC + 64:(r + 1) * DKC]
        bv_pad = np.zeros((1, 130), np.float16)
        bv_pad[0, 0:64] = np.asarray(bv, np.float16)[r * DKC:r * DKC + 64]
        bv_pad[0, 65:129] = np.asarray(bv, np.float16)[r * DKC + 64:
                                                       (r + 1) * DKC]
        bv_pad[0, 64] = 1.0
        bv_pad[0, 129] = 1.0
        in_maps.append({
            "xqT": q, "xkT": k, "xvT": v,
            "wq": np.ascontiguousarray(Wq[:, sl]),
            "wk": np.ascontiguousarray(Wk[:, sl]),
            "wv": wv_pad,
            "wo": Wo,
            "bq": np.ascontiguousarray(np.asarray(bq, np.float32)[sl, None]),
            "bk": np.ascontiguousarray(np.asarray(bk, np.float32)[sl, None]),
            "bv": bv_pad,
            "bo": bo_h,
        })

    res = run_bass_kernel_spmd(nc, in_maps, list(range(NCORES)), trace=trace)
    _cache["last_results"] = res

    out = np.empty((B, S, D), np.float32)
    for c in range(NCORES):
        o = res.results[c]["out"]
        for b in range(B):
            out[b, c * TOKB:(c + 1) * TOKB] = o[b * TOKB:(b + 1) * TOKB]
    return out


# revision 20
# speedup vs baseline: 1.0391x; 1.0255x over previous
"""Multi-head attention (B=2, S=2048, D=1024, H=16) on 8 NeuronCores.

Megatron tensor parallelism: core r owns heads 2r, 2r+1 (a 128-wide
slice of D). Wq/Wk/Wv column-parallel; output projection token-parallel
via one AllToAll per batch (128x256 fp16 blocks, normalization done
sender-side so the receive path feeds matmuls directly).

Schedule: the attention inner loop is ACT(exp)-bound. Each key tile
emits one [128,1024] two-bank score psum (two 512-col matmuls sharing
the stationary kT slice), ONE [128,1024] exp ACT, and two attnV
matmuls, software-pipelined with lookahead 1 so the PE always has a
score matmul in flight while ACT drains. Softmax normalization:
reciprocal of the ones-row sums (DVE) -> gpsimd partition_broadcast ->
fused psum*recip multiply (DVE), so the PE and ACT never touch it.
Batch-0 output-projection matmuls are drip-fed one per key tile into
batch-1's attention slack. A tiny warmup AllToAll absorbs launch skew.

DMA queues: inputs/weights/receive/stores on sync (HWDGE), staging on
vector, broadcasts + collectives on gpsimd.
"""

import sys

sys.path.insert(0, "/opt/trn_rl_repo")

import numpy as np

B, S, D, H, DK = 2, 2048, 1024, 16, 64
NCORES = 8
TOK = B * S            # 4096
DKC = D // NCORES      # 128 = 2 heads per core
TOKB = S // NCORES     # 256 tokens per core per batch
KT = D // 128          # 8 contraction tiles
SKT = S // 128         # 16 key tiles per batch

_cache = {}


def _build():
    from contextlib import ExitStack

    from concourse import bacc
    import concourse.mybir as mybir
    import concourse.tile as tile

    f32 = mybir.dt.float32
    f16 = mybir.dt.float16
    Act = mybir.ActivationFunctionType

    nc = bacc.Bacc(
        "TRN2", target_bir_lowering=False, debug=False,
        enable_asserts=False, num_devices=NCORES,
    )

    xqT = nc.dram_tensor("xqT", [D, TOK], f16, kind="ExternalInput").ap()
    xkT = nc.dram_tensor("xkT", [D, TOK], f16, kind="ExternalInput").ap()
    xvT = nc.dram_tensor("xvT", [D, TOK], f16, kind="ExternalInput").ap()
    wq = nc.dram_tensor("wq", [D, DKC], f16, kind="ExternalInput").ap()
    wk = nc.dram_tensor("wk", [D, DKC], f16, kind="ExternalInput").ap()
    wv = nc.dram_tensor("wv", [D, 130], f16, kind="ExternalInput").ap()
    wo = nc.dram_tensor("wo", [D, D], f16, kind="ExternalInput").ap()
    bq = nc.dram_tensor("bq", [DKC, 1], f32, kind="ExternalInput").ap()
    bk = nc.dram_tensor("bk", [DKC, 1], f32, kind="ExternalInput").ap()
    bv = nc.dram_tensor("bv", [1, 130], f16, kind="ExternalInput").ap()
    bo = nc.dram_tensor("bo", [1, D], f16, kind="ExternalInput").ap()
    out_ext = nc.dram_tensor("out", [2 * TOKB, D], f32, kind="ExternalOutput").ap()

    with tile.TileContext(nc) as tc, ExitStack() as ctx, \
            nc.allow_low_precision("fp16 matmul operands, fp32 psum accumulate"):
        wpool = ctx.enter_context(tc.tile_pool(name="w", bufs=1))
        xqpool = ctx.enter_context(tc.tile_pool(name="xq", bufs=8))
        xkpool = ctx.enter_context(tc.tile_pool(name="xk", bufs=8))
        xvpool = ctx.enter_context(tc.tile_pool(name="xv", bufs=8))
        qkpool = ctx.enter_context(tc.tile_pool(name="qk", bufs=1))
        vpool = ctx.enter_context(tc.tile_pool(name="v", bufs=1))
        ptpool = ctx.enter_context(tc.tile_pool(name="pt", bufs=4))
        lnpool = ctx.enter_context(tc.tile_pool(name="ln", bufs=1))
        spool = ctx.enter_context(tc.tile_pool(name="sum", bufs=2))
        rbpool = ctx.enter_context(tc.tile_pool(name="rb", bufs=2))
        rpool = ctx.enter_context(tc.tile_pool(name="recv", bufs=1))
        opool = ctx.enter_context(tc.tile_pool(name="o", bufs=2))
        ps_sc = ctx.enter_context(tc.tile_pool(name="pssc", bufs=2, space="PSUM"))
        ps_acc = ctx.enter_context(tc.tile_pool(name="psacc", bufs=2, space="PSUM"))
        dram = ctx.enter_context(tc.tile_pool(name="dram", bufs=1, space="DRAM"))

        # ---- warmup collective: absorb launch skew + link setup ----
        wusrc = dram.tile([8, 16], f32, tag="wusrc")
        wudst = dram.tile([8, 16], f32, tag="wudst")
        wuinit = wpool.tile([8, 16], f32, tag="wuinit")
        nc.gpsimd.memset(wuinit[:], 0.0)
        nc.sync.dma_start(wusrc[:], wuinit[:])
        nc.gpsimd.collective_compute(
            "AllToAll", mybir.AluOpType.bypass,
            replica_groups=[list(range(NCORES))],
            ins=[wusrc.opt()], outs=[wudst.opt()],
        )

        # ---- weights: consolidated DMAs on the scalar queue so input
        # loads own the sync queue from t=0 ----
        wq_t = wpool.tile([128, KT * DKC], f16, tag="wq")
        nc.scalar.dma_start(wq_t[:].rearrange("p (k m) -> p k m", k=KT),
                            wq.rearrange("(k p) m -> p k m", k=KT))
        wk_t = wpool.tile([128, KT * DKC], f16, tag="wk")
        nc.scalar.dma_start(wk_t[:].rearrange("p (k m) -> p k m", k=KT),
                            wk.rearrange("(k p) m -> p k m", k=KT))
        wv_t = wpool.tile([128, KT * 130], f16, tag="wv")
        nc.scalar.dma_start(wv_t[:].rearrange("p (k m) -> p k m", k=KT),
                            wv.rearrange("(k p) m -> p k m", k=KT))
        wo_t = wpool.tile([128, KT * D], f16, tag="wo")
        nc.scalar.dma_start(wo_t[:].rearrange("p (k m) -> p k m", k=KT),
                            wo.rearrange("(k p) m -> p k m", k=KT))
        bq_t = wpool.tile([DKC, 1], f32, tag="bq")
        nc.scalar.dma_start(bq_t[:], bq[:])
        bk_t = wpool.tile([DKC, 1], f32, tag="bk")
        nc.scalar.dma_start(bk_t[:], bk[:])
        bv_t = wpool.tile([1, 130], f16, tag="bv")
        nc.scalar.dma_start(bv_t[:], bv[:])
        bvb = wpool.tile([128, 130], f16, tag="bvb")
        nc.gpsimd.partition_broadcast(bvb[:], bv_t[:])
        bo_t = wpool.tile([1, D], f16, tag="bo")
        nc.scalar.dma_start(bo_t[:], bo[:])
        bob = wpool.tile([128, D], f16, tag="bob")
        nc.gpsimd.partition_broadcast(bob[:], bo_t[:])

        # ---- persistent per-batch tiles ----
        a2a_src = [dram.tile([NCORES * 128, TOKB], f16, tag=f"a2asrc{b}",
                             name=f"a2asrc{b}") for b in range(B)]
        a2a_dst = [dram.tile([NCORES * 128, TOKB], f16, tag=f"a2adst{b}",
                             name=f"a2adst{b}") for b in range(B)]
        ln_t = [lnpool.tile([128, S], f16, tag=f"ln{b}", name=f"ln{b}")
                for b in range(B)]
        rtile = [rpool.tile([128, KT * TOKB], f16, tag=f"r{b}",
                            name=f"r{b}") for b in range(B)]
        v_tiles = [[None] * SKT for _ in range(B)]
        qT = [None, None]
        kT = [None, None]

        GROUPS = [(sqg, h) for sqg in (0, 1) for h in (0, 1)]

        def emit_inputs(b, v_first=False):
            t0 = b * S
            xq_l, xk_l, xv_l = [], [], []
            order = ((xvT, xvpool, xv_l), (xqT, xqpool, xq_l),
                     (xkT, xkpool, xk_l)) if v_first else \
                    ((xqT, xqpool, xq_l), (xkT, xkpool, xk_l),
                     (xvT, xvpool, xv_l))
            for src, pool, lst in order:
                for k in range(KT):
                    t = pool.tile([128, S], f16, tag="xt")
                    nc.sync.dma_start(t[:], src[k * 128:(k + 1) * 128,
                                                t0:t0 + S])
                    lst.append(t)
            return xq_l, xk_l, xv_l

        def emit_qk_proj(b, xq_l, xk_l):
            qT[b] = qkpool.tile([128, S], f16, tag=f"qT{b}", name=f"qT{b}")
            kT[b] = qkpool.tile([128, S], f16, tag=f"kT{b}", name=f"kT{b}")
            for x_l, w_t, bias_t, dst in ((xq_l, wq_t, bq_t, qT[b]),
                                          (xk_l, wk_t, bk_t, kT[b])):
                for blk in range(4):
                    if blk % 2 == 0:
                        ps = ps_sc.tile([128, 1024], f32, tag="sc")
                    half = ps[:, (blk % 2) * 512:(blk % 2) * 512 + 512]
                    for k in range(KT):
                        nc.tensor.matmul(
                            half, lhsT=w_t[:, k * DKC:(k + 1) * DKC],
                            rhs=x_l[k][:, blk * 512:(blk + 1) * 512],
                            start=(k == 0), stop=(k == KT - 1),
                        )
                    nc.vector.tensor_scalar_add(
                        dst[:, blk * 512:(blk + 1) * 512], half,
                        bias_t[:, 0:1])

        def emit_v_proj(b, xv_l):
            for mi in range(SKT):
                if mi % 2 == 0:
                    ps = ps_sc.tile([128, 1024], f32, tag="sc")
                sub = ps[:, (mi % 2) * 512:(mi % 2) * 512 + 130]
                for k in range(KT):
                    nc.tensor.matmul(
                        sub, lhsT=xv_l[k][:, mi * 128:(mi + 1) * 128],
                        rhs=wv_t[:, k * 130:(k + 1) * 130],
                        start=(k == 0), stop=(k == KT - 1),
                    )
                vt = vpool.tile([128, 130], f16, tag=f"v{b}_{mi}")
                nc.vector.tensor_add(vt[:], sub, bvb[:])
                v_tiles[b][mi] = vt

        def emit_score(b, g, sk):
            sqg, h = GROUPS[g]
            hp = h * 64
            sp = ps_sc.tile([128, 1024], f32, tag="sc")
            for j in (0, 1):
                nc.tensor.matmul(
                    sp[:, j * 512:(j + 1) * 512],
                    lhsT=kT[b][hp:hp + 64, sk * 128:(sk + 1) * 128],
                    rhs=qT[b][hp:hp + 64,
                              (sqg * 2 + j) * 512:(sqg * 2 + j + 1) * 512],
                    start=True, stop=True,
                )
            return sp

        def emit_group_tail(b, g, acc):
            """Sender-side softmax normalization + staging for group g."""
            sqg, h = GROUPS[g]
            hp = h * 64
            sf = spool.tile([1, 1024], f32, tag="sf")
            nc.vector.tensor_copy(sf[:], acc[64:65, :])
            # NOTE: reciprocal_approx_fast misbehaves on HW when its input
            # AP starts at a nonzero base partition, so feed it base-0 SBUF.
            rf = spool.tile([1, 1024], f32, tag="rf")
            nc.vector.reciprocal_approx_fast(rf[:], sf[:])
            rb = rbpool.tile([64, 1024], f32, tag="rb")
            nc.gpsimd.partition_broadcast(rb[:], rf[:])
            nc.vector.tensor_mul(
                ln_t[b][hp:hp + 64, sqg * 1024:(sqg + 1) * 1024],
                acc[0:64, :], rb[:])
            if h == 1:
                nc.gpsimd.dma_start(
                    a2a_src[b][sqg * 512:(sqg + 1) * 512, :]
                    .rearrange("(c p) t -> p c t", c=4),
                    ln_t[b][:, sqg * 1024:(sqg + 1) * 1024]
                    .rearrange("p (c t) -> p c t", c=4))

        def qkproj_thunks(b, xq_l, xk_l):
            """q/k projection as filler thunks; psum gens allocated
            lazily from the acc tag so rotation follows emission order."""
            qT[b] = qkpool.tile([128, S], f16, tag=f"qT{b}", name=f"qT{b}")
            kT[b] = qkpool.tile([128, S], f16, tag=f"kT{b}", name=f"kT{b}")
            thunks = []
            for x_l, w_t, bias_t, dst in ((xq_l, wq_t, bq_t, qT[b]),
                                          (xk_l, wk_t, bk_t, kT[b])):
                for pair in range(2):
                    cell = {}
                    for blk in (pair * 2, pair * 2 + 1):
                        for k in range(KT):
                            def mm(blk=blk, k=k, cell=cell, x_l=x_l,
                                   w_t=w_t):
                                if "ps" not in cell:
                                    cell["ps"] = ps_acc.tile(
                                        [128, 1024], f32, tag="acc",
                                        name="pjps")
                                nc.tensor.matmul(
                                    cell["ps"][:, (blk % 2) * 512:
                                               (blk % 2) * 512 + 512],
                                    lhsT=w_t[:, k * DKC:(k + 1) * DKC],
                                    rhs=x_l[k][:, blk * 512:(blk + 1) * 512],
                                    start=(k == 0), stop=(k == KT - 1),
                                )
                            thunks.append(mm)

                        def bias(blk=blk, cell=cell, bias_t=bias_t, dst=dst):
                            nc.vector.tensor_scalar_add(
                                dst[:, blk * 512:(blk + 1) * 512],
                                cell["ps"][:, (blk % 2) * 512:
                                           (blk % 2) * 512 + 512],
                                bias_t[:, 0:1])
                        thunks.append(bias)
            return thunks

        def outproj_mms(b, m2):
            """Thunks for one out-proj block; psum allocated at first call
            so the pool rotation follows actual emission order."""
            cell = {}

            def mk(n2, k):
                def mm():
                    if "op" not in cell:
                        cell["op"] = ps_acc.tile([128, 1024], f32, tag="acc", name="op")
                    nc.tensor.matmul(
                        cell["op"][:, n2 * 512:(n2 + 1) * 512],
                        lhsT=rtile[b][:, k * TOKB + m2 * 128:
                                      k * TOKB + (m2 + 1) * 128],
                        rhs=wo_t[:, k * D + n2 * 512:k * D + (n2 + 1) * 512],
                        start=(k == 0), stop=(k == KT - 1),
                    )
                return mm

            thunks = [mk(n2, k) for n2 in (0, 1) for k in range(KT)]

            def fin():
                ot = opool.tile([128, 1024], f32, tag="ot")
                nc.vector.tensor_add(ot[:], cell["op"][:], bob[:])
                nc.sync.dma_start(
                    out_ext[b * TOKB + m2 * 128:b * TOKB + (m2 + 1) * 128, :],
                    ot[:])
            thunks.append(fin)
            return thunks

        def emit_attention(b, pe_filler):
            units = [(g, sk) for g in range(4) for sk in range(SKT)]
            sps = {0: emit_score(b, *units[0]), 1: emit_score(b, *units[1])}
            accs = {}
            for i, (g, sk) in enumerate(units):
                if g not in accs:
                    accs[g] = ps_acc.tile([128, 1024], f32, tag="acc", name="acc")
                sqg, h = GROUPS[g]
                pt = ptpool.tile([128, 1024], f16, tag="pt")
                nc.scalar.activation(pt[:], sps.pop(i)[:], Act.Exp,
                                     scale=0.125)
                for j in (0, 1):
                    nc.tensor.matmul(
                        accs[g][0:65, j * 512:(j + 1) * 512],
                        lhsT=v_tiles[b][sk][:, h * 65:h * 65 + 65],
                        rhs=pt[:, j * 512:(j + 1) * 512],
                        start=(sk == 0), stop=(sk == SKT - 1),
                    )
                if i + 2 < len(units):
                    sps[i + 2] = emit_score(b, *units[i + 2])
                # drip out-proj work into the ACT-bound slack; start after
                # the first group's accumulator has been fully read so the
                # borrowed ps_acc generations never stall the PE queue
                if pe_filler and i >= 17:
                    pe_filler.pop(0)()
                if sk == SKT - 1:
                    emit_group_tail(b, g, accs.pop(g))

        def emit_recv(b):
            nc.sync.dma_start(
                rtile[b][:].rearrange("p (k t) -> p k t", k=KT),
                a2a_dst[b].rearrange("(k p) t -> p k t", k=KT))

        # ================= emission =================
        def emit_a2a(b):
            nc.gpsimd.collective_compute(
                "AllToAll", mybir.AluOpType.bypass,
                replica_groups=[list(range(NCORES))],
                ins=[a2a_src[b].opt()], outs=[a2a_dst[b].opt()],
            )

        xq0, xk0, xv0 = emit_inputs(0, v_first=True)
        xq1, xk1, xv1 = emit_inputs(1)
        emit_v_proj(0, xv0)
        emit_qk_proj(0, xq0, xk0)
        f0 = qkproj_thunks(1, xq1, xk1)
        emit_attention(0, f0)
        while f0:
            f0.pop(0)()
        emit_a2a(0)
        emit_recv(0)
        emit_v_proj(1, xv1)
        f1 = outproj_mms(0, 0) + outproj_mms(0, 1)
        emit_attention(1, f1)
        while f1:
            f1.pop(0)()
        emit_a2a(1)

        # ---- tail: batch-1 receive + output projection ----
        emit_recv(1)
        for m2 in (0, 1):
            for f in outproj_mms(1, m2):
                f()

    nc.compile()
    return nc


def _get_nc():
    if "nc" not in _cache:
        _cache["nc"] = _build()
    return _cache["nc"]


def kernel(query, key, value, Wq, bq, Wk, bk, Wv, bv, Wo, bo, trace=False):
    from concourse.bass_utils import run_bass_kernel_spmd

    nc = _get_nc()

    q = np.ascontiguousarray(
        np.asarray(query, np.float32).reshape(TOK, D).T.astype(np.float16))
    k = np.ascontiguousarray(
        np.asarray(key, np.float32).reshape(TOK, D).T.astype(np.float16))
    v = np.ascontiguousarray(
        np.asarray(value, np.float32).reshape(TOK, D).T.astype(np.float16))
    Wq = np.asarray(Wq, np.float16)
    Wk = np.asarray(Wk, np.float16)
    Wv = np.asarray(Wv, np.float16)
    Wo = np.ascontiguousarray(np.asarray(Wo, np.float16))
    bo_h = np.ascontiguousarray(np.asarray(bo, np.float16)[None, :])

    in_maps = []
    for r in range(NCORES):
        sl = slice(r * DKC, (r + 1) * DKC)
        # wv/bv padded to 130 cols: col 64 and 129 carry the softmax-sum
        # ones column (weight 0, bias 1).
        wv_pad = np.zeros((D, 130), np.float16)
        wv_pad[:, 0:64] = Wv[:, r * DKC:r * DKC + 64]
        wv_pad[:, 65:129] = Wv[:, r * DKC + 64:(r + 1) * DKC]
        bv_pad = np.zeros((1, 130), np.float16)
        bv_pad[0, 0:64] = np.asarray(bv, np.float16)[r * DKC:r * DKC + 64]
        bv_pad[0, 65:129] = np.asarray(bv, np.float16)[r * DKC + 64:
                                                       (r + 1) * DKC]
        bv_pad[0, 64] = 1.0
        bv_pad[0, 129] = 1.0
        in_maps.append({
            "xqT": q, "xkT": k, "xvT": v,
            "wq": np.ascontiguousarray(Wq[:, sl]),
            "wk": np.ascontiguousarray(Wk[:, sl]),
            "wv": wv_pad,
            "wo": Wo,
            "bq": np.ascontiguousarray(np.asarray(bq, np.float32)[sl, None]),
            "bk": np.ascontiguousarray(np.asarray(bk, np.float32)[sl, None]),
            "bv": bv_pad,
            "bo": bo_h,
        })

    res = run_bass_kernel_spmd(nc, in_maps, list(range(NCORES)), trace=trace)
    _cache["last_results"] = res

    out = np.empty((B, S, D), np.float32)
    for c in range(NCORES):
        o = res.results[c]["out"]
        for b in range(B):
            out[b, c * TOKB:(c + 1) * TOKB] = o[b * TOKB:(b + 1) * TOKB]
    return out
